# revision 39
# baseline (speedup 1.0000x reference)
"""Trainium2 Bass kernel for fused multi-head attention (16 heads, d=64,
b=2, n=2048, h=1024) across 8 NeuronCores.

Sharding: 2 heads x BOTH batches per core (core c owns heads {2c, 2c+1}).
With every core holding a slice of every batch, the post-attention
Ulysses AllToAll carries no batch duplication: each 512-row piece is a
256KB exchange whose every shard is useful, and the received shards feed
the output projection directly (no mask-combine). Each core then runs
the output projection for its 128-row slice of batch c//4 over the full
1024-dim contraction.

Schedule: one software-pipelined stream, TensorE-throughput-bound at the
power-capped PE clock (~263ns per 512-wide matmul). Staging is arranged
so only batch-0 x + the K/Q weight column groups + the low cos/sin
halves gate the first exp; batch-1 x and the output weights trail behind.
Attention positions run (b,qc) = (0,0),(0,1),(1,0),(0,2),(1,1),(0,3),
(1,2),(1,3); projections and V chunks are interleaved just-in-time into
the attention matmul stream (grab 512-wide psums, then rotate; pairs
kept adjacent so the 2-buffer PSUM pool never head-of-line blocks the PE
queue). Each position's attn_out^T (+denominator row from a ones-column
in V) is copied out of PSUM immediately; the normalization chain runs at
kc2-6 of the next position; AllToAll pieces trigger as soon as both
halves are normalized. All collective-gated work (recv DMAs + output
projections) is priority-pinned to the tail so launch/collective skew
can never stall the attention pipeline. ScalarE runs exp exclusively.
"""

import sys

if "/opt/trn_rl_repo" not in sys.path:
    sys.path.insert(0, "/opt/trn_rl_repo")

import numpy as np
import ml_dtypes

import concourse.bass as bass
import concourse.mybir as mybir
import concourse.tile as tile
from concourse import bacc
from concourse.bass import ts
from concourse.bass_utils import run_bass_kernel_spmd

BF16 = mybir.dt.bfloat16
F32 = mybir.dt.float32
ADD = mybir.AluOpType.add
MULT = mybir.AluOpType.mult
BYPASS = mybir.AluOpType.bypass
EXP = mybir.ActivationFunctionType.Exp

HEADS, D, H, N, B = 16, 64, 1024, 2048, 2
NC_ = 8
LH = 2            # local heads per core (one pair, both batches)
KC = 16           # k chunks of 128 over n=2048
QC = 4            # q chunks of 512 over n=2048 (= AllToAll pieces)
LVW = LH * 65     # 130: local v-aug width
LQK = LH * D      # 128 local q (or k) columns
GK, GQ = 0, 128   # wqk column groups (host-reordered): [K | Q]


def build_nc():
    nc = bacc.Bacc("TRN2", target_bir_lowering=False, debug=False, num_devices=NC_)

    # x for both batches, b-major columns: [h, b*N + pos]
    xT = nc.declare_dram_parameter("xT", [H, B * N], BF16, isOutput=False)
    wqk = nc.declare_dram_parameter("wqk", [H, 2 * LQK], BF16, isOutput=False)
    wv = nc.declare_dram_parameter("wv", [H, LVW], BF16, isOutput=False)
    wout = nc.declare_dram_parameter("wout", [H, H], BF16, isOutput=False)
    cos2 = nc.declare_dram_parameter("cos2", [64, N], BF16, isOutput=False)
    # sinm[p] = sin value read at SOURCE partition p during the shuffle:
    # p%64 < 32 -> +sin[p%64+32], else -sin[p%64-32]
    sinm = nc.declare_dram_parameter("sinm", [64, N], BF16, isOutput=False)
    out = nc.declare_dram_parameter("out", [QC, 128, H], BF16, isOutput=True)

    with tile.TileContext(nc) as tc:
        with (
            tc.tile_pool(name="dram", bufs=1, space="DRAM") as dram,
            tc.tile_pool(name="sb", bufs=1) as sb,
            tc.tile_pool(name="sbw", bufs=1) as sbw,
            tc.tile_pool(name="psum", bufs=2, space="PSUM") as ps,
        ):
            a2a_in = [dram.tile([8, 128, 128], BF16, name=f"ain{i}")
                      for i in range(QC)]
            a2a_out = [dram.tile([8, 128, 128], BF16, name=f"aout{i}")
                       for i in range(QC)]

            # warmup collective first: absorbs the one-time CC barrier
            # under the staging/projection prologue
            warm_in = dram.tile([8, 128], BF16, name="warm_in")
            warm_out = dram.tile([8, 128], BF16, name="warm_out")
            warm_sb = sbw.tile([1, 128], BF16)
            nc.vector.memset(warm_sb[:, :], 0.0)
            nc.scalar.dma_start(warm_in[0:1, :], warm_sb[:, :])
            nc.gpsimd.collective_compute(
                "AllToAll", BYPASS, replica_groups=[list(range(8))],
                ins=[warm_in.opt()], outs=[warm_out.opt()])

            # ---- staging: batch-0 x + K/Q groups + low cos/sin halves
            # gate the first exp; batch-1 x and wout trail ----
            xt_sb = sbw.tile([128, B * 8 * N], BF16)   # [p, b, hk, pos]
            wqk_sb = sbw.tile([128, 8 * 2 * LQK], BF16)
            wv_sb = sbw.tile([128, 8 * LVW], BF16)
            wout_sb = sbw.tile([128, 8 * H], BF16)
            cos2_sb = sbw.tile([128, N], BF16)
            sinm_sb = sbw.tile([128, N], BF16)
            ones_sb = sbw.tile([1, D], BF16)

            wqk_src = wqk.rearrange("(k p) c -> p k c", p=128)
            wqk_dst = wqk_sb.rearrange("p (k c) -> p k c", c=2 * LQK)

            def wqk_grp(eng, g):
                eng.dma_start(wqk_dst[:, :, g:g + 128],
                              wqk_src[:, :, g:g + 128])

            xt_src = xT.rearrange("(k p) (b c) -> p b k c", p=128, b=B)
            xt_dst = xt_sb.rearrange("p (b k c) -> p b k c", b=B, c=N)

            # x rides the SWDGE path (gpsimd): one big DMA splits across
            # all 16 SDMA engines while the HWDGE queues carry the small
            # weights in parallel
            nc.gpsimd.dma_start(xt_dst[:, 0], xt_src[:, 0])
            wqk_grp(nc.sync, GK)
            nc.sync.dma_start(cos2_sb[0:64, 0:1024], cos2[:, 0:1024])
            nc.sync.dma_start(sinm_sb[0:64, 0:1024], sinm[:, 0:1024])
            wqk_grp(nc.scalar, GQ)
            nc.scalar.dma_start(cos2_sb[0:64, 1024:2048], cos2[:, 1024:2048])
            nc.scalar.dma_start(sinm_sb[0:64, 1024:2048], sinm[:, 1024:2048])
            nc.scalar.dma_start(
                wv_sb.rearrange("p (k c) -> p k c", c=LVW),
                wv.rearrange("(k p) c -> p k c", p=128))
            # batch-1 x trails on the same SWDGE path
            nc.gpsimd.dma_start(xt_dst[:, 1], xt_src[:, 1])
            nc.vector.memset(ones_sb[:, :], 1.0)
            # duplicate cos/sin to partitions 64-127 (per-half, on DVE)
            for lo in (0, 1024):
                nc.vector.tensor_copy(cos2_sb[64:128, lo:lo + 1024],
                                      cos2_sb[0:64, lo:lo + 1024])
                nc.vector.tensor_copy(sinm_sb[64:128, lo:lo + 1024],
                                      sinm_sb[0:64, lo:lo + 1024])

            kt_rot = sb.tile([128, B * N], BF16)   # [batch b at b*N][n]
            qt_rot = sb.tile([128, B * N], BF16)
            vt_all = sb.tile([128, B * KC * LVW], BF16)
            # attn^T laid out [qc][b][row-block r][row-in-block]: shard
            # j = 4b+r of piece qc is the contiguous span (qc, b, r)
            attn_sb = sb.tile([128, B * N], BF16)
            attn4 = attn_sb.rearrange("p (q b r x) -> p q b r x", q=QC, b=B,
                                      x=128)

            # per-head ones columns of v-aug, set once
            nc.vector.memset(
                vt_all.rearrange("p (g e) -> p g e", e=65)[:, :, 64:65], 1.0)


            def proj_group(col0, b, sc):
                p = ps.tile([128, 512], F32, tag="b", name="pp")
                for hk in range(8):
                    nc.tensor.matmul(
                        p[:, :],
                        lhsT=wqk_sb[:, hk * 2 * LQK + col0:][:, :128],
                        rhs=xt_sb[:, (b * 8 + hk) * N + sc * 512:][:, :512],
                        start=(hk == 0),
                        stop=(hk == 7),
                    )
                return p

            def rotary_apply(psums, dst, pos0, W):
                """Rotary on a [128, W] span covering positions
                [pos0, pos0+W): stage psums to bf16 (DVE), then the
                partition-swapped sin multiply + cos multiply + add."""
                stage = sb.tile([128, 1024], BF16, tag="stg", bufs=2, name="stg")
                for i, p in enumerate(psums):
                    nc.vector.tensor_copy(stage[:, ts(i, 512)], p[:, :])
                tmp = sb.tile([128, 1024], BF16, tag="rta", bufs=2, name="rta")
                tmp2 = sb.tile([128, 1024], BF16, tag="rtb", bufs=2, name="rtb")
                sl = sinm_sb[:, pos0:pos0 + W]
                cl = cos2_sb[:, pos0:pos0 + W]
                for hh in (0, 64):
                    nc.vector.tensor_tensor(
                        tmp[hh:hh + 32, :W], stage[hh + 32:hh + 64, :W],
                        sl[hh + 32:hh + 64, :], MULT)
                    nc.vector.tensor_tensor(
                        tmp[hh + 32:hh + 64, :W], stage[hh:hh + 32, :W],
                        sl[hh:hh + 32, :], MULT)
                nc.vector.tensor_tensor(tmp2[:, :W], stage[:, :W], cl, MULT)
                nc.vector.tensor_tensor(dst, tmp2[:, :W], tmp[:, :W], ADD)

            def v_chunk(b, rc):
                p = ps.tile([128, LVW], F32, tag="b", name="vp")
                for hk in range(8):
                    nc.tensor.matmul(
                        p[:, :],
                        lhsT=xt_sb[:, (b * 8 + hk) * N + rc * 128:][:, :128],
                        rhs=wv_sb[:, ts(hk, LVW)],
                        start=(hk == 0),
                        stop=(hk == 7),
                    )
                nc.vector.tensor_copy(
                    vt_all[:, (b * KC + rc) * LVW:][:, :LVW].rearrange(
                        "p (h e) -> p h e", e=65)[:, :, 0:64],
                    p.rearrange("p (h e) -> p h e", e=65)[:, :, 0:64])

            def _av_mm(e, av0, av1, b, kc):
                base = (b * KC + kc) * LVW
                nc.tensor.matmul(
                    av0[:, :], lhsT=vt_all[:, base:][:, :65],
                    rhs=e[:, 0:512], start=(kc == 0), stop=(kc == KC - 1))
                nc.tensor.matmul(
                    av1[:, :], lhsT=vt_all[:, base + 65:][:, :65],
                    rhs=e[:, 512:1024], start=(kc == 0), stop=(kc == KC - 1))

            def finish_copy(av0, av1):
                # move attn_out^T (+denominator row 64) out of PSUM right
                # away so the next position's AV accumulation can reuse the
                # banks; also stage both denominator rows at partition 0 for
                # the broadcast matmul. Normalization itself runs later.
                a0 = sb.tile([65, 512], BF16, tag="avs", bufs=4, name="a0")
                a1 = sb.tile([65, 512], BF16, tag="avs", bufs=4, name="a1")
                nc.vector.tensor_copy(a0[:, :], av0[:, :])
                nc.vector.tensor_copy(a1[:, :], av1[:, :])
                ad = sb.tile([1, 1024], BF16, tag="adn", bufs=2, name="ad")
                nc.vector.tensor_copy(ad[:, 0:512], a0[64:65, :])
                nc.vector.tensor_copy(ad[:, 512:1024], a1[64:65, :])
                return a0, a1, ad

            def finish_norm(qc, b, a0, a1, ad):
                b_ps = ps.tile([128, 512], F32, tag="b", name="b_ps")
                nc.tensor.matmul(b_ps[0:64, :], lhsT=ones_sb[:, :],
                                 rhs=ad[:, 0:512], start=True, stop=True,
                                 tile_position=(0, 0))
                nc.tensor.matmul(b_ps[64:128, :], lhsT=ones_sb[:, :],
                                 rhs=ad[:, 512:1024], start=True, stop=True,
                                 tile_position=(0, 64))
                bd_sb = sb.tile([128, 512], F32, tag="bsd", bufs=2, name="bd_sb")
                nc.vector.tensor_copy(bd_sb[:, :], b_ps[:, :])
                b_sb = sb.tile([128, 512], F32, tag="bsb", bufs=2, name="b_sb")
                nc.vector.reciprocal_approx_fast(out=b_sb[:, :], in_=bd_sb[:, :])
                # both TT inputs must share a base partition: bring head B's
                # inv-denominators down to partitions 0-63
                b_lo = sb.tile([64, 512], F32, tag="blo", bufs=2, name="b_lo")
                nc.vector.tensor_copy(b_lo[:, :], b_sb[64:128, :])
                dst = attn4[:, qc, b, :, :]  # [128, 4, 128]
                b3 = b_sb.rearrange("p (j x) -> p j x", x=128)
                bl3 = b_lo.rearrange("p (j x) -> p j x", x=128)
                nc.vector.tensor_tensor(dst[0:64], a0[0:64, :].rearrange(
                    "p (j x) -> p j x", x=128), b3[0:64], MULT)
                nc.vector.tensor_tensor(dst[64:128], a1[0:64, :].rearrange(
                    "p (j x) -> p j x", x=128), bl3[:, :, :], MULT)

            def a2a_send(qc, b):
                # shard j=4b+r carries my 2 heads for (batch b, row block r)
                # of piece qc; b=None sends both batches in one DMA
                src = attn4[:, qc, :, :, :]          # [128, 2, 4, 128]
                d = a2a_in[qc].rearrange("(b r) p x -> p b r x", b=B)
                if b is not None:
                    src = src[:, b:b + 1, :, :]
                    d = d[:, b:b + 1, :, :]
                nc.sync.dma_start(d, src)

            def a2a_go(qc):
                nc.gpsimd.collective_compute(
                    "AllToAll", BYPASS, replica_groups=[list(range(8))],
                    ins=[a2a_in[qc].opt()], outs=[a2a_out[qc].opt()])

            def emit_a2a(qc):
                a2a_send(qc, None)
                a2a_go(qc)

            # tail-only: shard i of a2a_out = peer i's heads {2i, 2i+1} for
            # my 128 rows -> directly the outproj stationary operand
            def outproj_recv(qc):
                att_r = sb.tile([128, 8 * 128], BF16, tag="attr", bufs=2,
                                name="att_r")
                nc.gpsimd.dma_start(
                    att_r.rearrange("p (i x) -> p i x", i=8),
                    a2a_out[qc].rearrange("i p x -> p i x"))
                return att_r

            def outproj_mm(qc, att_r, nh):
                g3 = att_r.rearrange("p (c x) -> p c x", x=128)
                o_ps = ps.tile([128, 512], F32, tag="b", name="o_ps")
                for hc in range(8):
                    nc.tensor.matmul(
                        o_ps[:, :],
                        lhsT=g3[:, hc, :],
                        rhs=wout_sb[:, hc * H + nh * 512:][:, :512],
                        start=(hc == 0),
                        stop=(hc == 7),
                    )
                ob = sb.tile([128, 512], BF16, tag="ob", bufs=3, name="ob")
                nc.vector.tensor_copy(ob[:, :], o_ps[:, :])
                # store via the faster SWDGE path; emitted before the next
                # piece's recv so a waiting recv never blocks completed data
                nc.gpsimd.dma_start(out[qc, :, ts(nh, 512)], ob[:, :])

            def att_pos(qc, b, hooks):
                qt_p = qt_rot[:, b * N + qc * 512:][:, :512]
                av0 = ps.tile([65, 512], F32, tag="av", name="av0")
                av1 = ps.tile([65, 512], F32, tag="av", name="av1")
                exps = []
                for kc in range(KC):
                    s_ps = ps.tile([128, 1024], F32, tag="s", name="s_ps")
                    nc.tensor.matmul(
                        s_ps[:, 0:512],
                        lhsT=kt_rot[0:64, b * N + kc * 128:][:, :128],
                        rhs=qt_p[0:64, :], start=True, stop=True,
                        tile_position=(0, 0))
                    nc.tensor.matmul(
                        s_ps[:, 512:1024],
                        lhsT=kt_rot[64:128, b * N + kc * 128:][:, :128],
                        rhs=qt_p[64:128, :], start=True, stop=True,
                        tile_position=(64, 0))
                    e = sb.tile([128, 1024], BF16, tag="exp", bufs=6, name="e")
                    nc.scalar.activation(e[:, :], s_ps[:, :], EXP, scale=0.125)
                    exps.append(e)
                    for f in hooks.get(kc, []):
                        f()
                    if kc > 0:
                        _av_mm(exps[kc - 1], av0, av1, b, kc - 1)
                _av_mm(exps[KC - 1], av0, av1, b, KC - 1)
                return finish_copy(av0, av1)

            # ---- prologue ----
            # gate of the first exp: batch-0 K low half + Q sc0 + rotary;
            # V chunks 0-2 (b0) fill the PE while rotary runs on DVE
            k0a = [proj_group(GK, 0, 0), proj_group(GK, 0, 1)]
            rotary_apply(k0a, kt_rot[:, 0:1024], 0, 1024)
            q0s0 = [proj_group(GQ, 0, 0)]
            rotary_apply(q0s0, qt_rot[:, 0:512], 0, 512)
            for rc in range(3):
                v_chunk(0, rc)

            # closure helpers for hook tables
            def mk(f, *a):
                return lambda: f(*a)

            grabs = {}

            def grab(key, col0, b, sc):
                def g():
                    grabs.setdefault(key, []).append(proj_group(col0, b, sc))
                return g

            def rot(key, dst_tile, b, half):
                def g():
                    rotary_apply(
                        grabs.pop(key),
                        dst_tile[:, b * N + half * 1024:][:, :1024],
                        half * 1024, 1024)
                return g

            def rot_half(key, pos0):
                # 512-wide rotary chunk of batch-0 Q
                rotary_apply(grabs.pop(key),
                             qt_rot[:, pos0:pos0 + 512], pos0, 512)

            pend = {}

            def norm(qc, b):
                def g():
                    a0, a1, ad = pend.pop((qc, b))
                    finish_norm(qc, b, a0, a1, ad)
                return g

            def wout_dma():
                nc.sync.dma_start(
                    wout_sb.rearrange("p (k c) -> p k c", c=H),
                    wout.rearrange("(k p) c -> p k c", p=128))

            SEQ = [
                # (b, qc, hooks); norm(qc, b) finalizes the PREVIOUS
                # position; grab-pairs stay adjacent (2-buffer PSUM pool)
                (0, 0, dict(list({
                    0: [grab("k0b", GK, 0, 2)],
                    1: [grab("k0b", GK, 0, 3)],
                    2: [rot("k0b", kt_rot, 0, 1), mk(v_chunk, 0, 3)],
                    6: [grab("q0s1", GQ, 0, 1), mk(v_chunk, 0, 7)],
                    7: [mk(rot_half, "q0s1", 512), mk(v_chunk, 0, 8)],
                }.items()) + [(kc, [mk(v_chunk, 0, kc + 1)])
                              for kc in (3, 4, 5)]
                             + [(kc, [mk(v_chunk, 0, kc + 1)])
                                for kc in range(8, 15)])),
                (0, 1, {
                    1: [grab("k1a", GK, 1, 0)],
                    3: [grab("k1a", GK, 1, 1)],
                    5: [rot("k1a", kt_rot, 1, 0)],
                    6: [norm(0, 0), wout_dma],
                    7: [grab("q1a", GQ, 1, 0)],
                    9: [grab("q1a", GQ, 1, 1)],
                    11: [rot("q1a", qt_rot, 1, 0)],
                    12: [mk(v_chunk, 1, 0), mk(v_chunk, 1, 1),
                         mk(v_chunk, 1, 2)],
                    13: [mk(v_chunk, 1, 3), mk(v_chunk, 1, 4),
                         mk(v_chunk, 1, 5)],
                    14: [grab("k1b", GK, 1, 2)],
                    15: [grab("k1b", GK, 1, 3)],
                }),
                (1, 0, dict(list({
                    0: [rot("k1b", kt_rot, 1, 1)],
                    2: [norm(1, 0)],
                    3: [grab("q0b", GQ, 0, 2)],
                    4: [grab("q0b", GQ, 0, 3)],
                    5: [rot("q0b", qt_rot, 0, 1), mk(v_chunk, 1, 6)],
                }.items()) + [(kc, [mk(v_chunk, 1, kc + 1)])
                              for kc in range(6, 15)])),
                (0, 2, {
                    1: [grab("q1b", GQ, 1, 2)],
                    3: [grab("q1b", GQ, 1, 3)],
                    5: [rot("q1b", qt_rot, 1, 1)],
                    6: [norm(0, 1)],
                    8: [mk(emit_a2a, 0)],
                }),
                (1, 1, {
                    2: [norm(2, 0)],
                }),
                (0, 3, {
                    2: [norm(1, 1)],
                    6: [mk(emit_a2a, 1)],
                }),
                (1, 2, {
                    2: [norm(3, 0)],
                }),
                (1, 3, {
                    2: [norm(2, 1)],
                    4: [mk(emit_a2a, 2)],
                    8: [mk(a2a_send, 3, 0)],
                }),
            ]
            for b, qc, hooks in SEQ:
                pend[(qc, b)] = att_pos(qc, b, hooks)

            # tail: final normalization + piece-3 collective, then ALL
            # output projections (priority-pinned so the scheduler cannot
            # hoist collective-gated work into the attention stream)
            with tc.high_priority():
                norm(3, 1)()
                a2a_send(3, 1)
            a2a_go(3)
            with tc.high_priority(offset=-10_000_000):
                for qc in range(QC):
                    ar = outproj_recv(qc)
                    outproj_mm(qc, ar, 0)
                    outproj_mm(qc, ar, 1)

    nc.finalize()
    return nc


_NC = None


def _get_nc():
    global _NC
    if _NC is None:
        _NC = build_nc()
    return _NC


def _bf16(a):
    return np.ascontiguousarray(a.astype(ml_dtypes.bfloat16))


def make_in_maps(x, rotary_emb, w_qkv, w_out):
    x = np.asarray(x, np.float32)
    rotary_emb = np.asarray(rotary_emb, np.float32)
    w_qkv = np.asarray(w_qkv, np.float32)
    w_out = np.asarray(w_out, np.float32)
    cosT = np.cos(rotary_emb).T.astype(np.float32)  # [64, N]
    sinT = np.sin(rotary_emb).T.astype(np.float32)
    sswp = np.concatenate([sinT[32:], -sinT[:32]], axis=0)
    cos2_a = _bf16(cosT)
    sinm_a = _bf16(sswp)
    wout_bf = _bf16(w_out)
    xT_a = _bf16(np.concatenate([x[0].T, x[1].T], axis=1))  # [H, B*N]
    in_maps = []
    for c in range(NC_):
        h0 = LH * c  # heads {2c, 2c+1}
        wq_loc = w_qkv[:, 64 * h0: 64 * h0 + LQK]
        wk_loc = w_qkv[:, H + 64 * h0: H + 64 * h0 + LQK]
        wv_loc = w_qkv[:, 2 * H + 64 * h0: 2 * H + 64 * h0 + LQK]
        wv_aug = np.zeros((H, LVW), np.float32)
        for j in range(LH):
            wv_aug[:, 65 * j: 65 * j + 64] = wv_loc[:, 64 * j: 64 * j + 64]
        # column-grouped wqk: [K | Q]
        wqk_g = np.concatenate([wk_loc, wq_loc], axis=1)
        in_maps.append({
            "xT": xT_a,
            "wqk": _bf16(wqk_g),
            "wv": _bf16(wv_aug),
            "wout": wout_bf,
            "cos2": cos2_a,
            "sinm": sinm_a,
        })
    return in_maps


def run(x, rotary_emb, w_qkv, w_out, trace=False, tmpdir=None):
    nc = _get_nc()
    in_maps = make_in_maps(x, rotary_emb, w_qkv, w_out)
    res = run_bass_kernel_spmd(nc, in_maps, list(range(NC_)), trace=trace,
                               tmpdir=tmpdir)
    full = np.empty((B, N, H), np.float32)
    for c in range(NC_):
        b, r = c // 4, c % 4
        piece = np.asarray(res.results[c]["out"]).astype(np.float32)
        for qc in range(QC):
            full[b, 512 * qc + 128 * r: 512 * qc + 128 * r + 128] = piece[qc]
    return full, res


def kernel(x, rotary_emb, w_qkv, w_out):
    full, _ = run(x, rotary_emb, w_qkv, w_out)
    return full


# revision 40
# speedup vs baseline: 1.0098x; 1.0098x over previous
"""Trainium2 Bass kernel for fused multi-head attention (16 heads, d=64,
b=2, n=2048, h=1024) across 8 NeuronCores.

Sharding: 2 heads x BOTH batches per core (core c owns heads {2c, 2c+1}).
With every core holding a slice of every batch, the post-attention
Ulysses AllToAll carries no batch duplication: each 512-row piece is a
256KB exchange whose every shard is useful, and the received shards feed
the output projection directly (no mask-combine). Each core then runs
the output projection for its 128-row slice of batch c//4 over the full
1024-dim contraction.

Schedule: one software-pipelined stream, TensorE-throughput-bound at the
power-capped PE clock (~263ns per 512-wide matmul). Staging is arranged
so only batch-0 x + the K/Q weight column groups + the low cos/sin
halves gate the first exp; batch-1 x and the output weights trail behind.
Attention positions run (b,qc) = (0,0),(0,1),(1,0),(0,2),(1,1),(0,3),
(1,2),(1,3); projections and V chunks are interleaved just-in-time into
the attention matmul stream (grab 512-wide psums, then rotate; pairs
kept adjacent so the 2-buffer PSUM pool never head-of-line blocks the PE
queue). Each position's attn_out^T (+denominator row from a ones-column
in V) is copied out of PSUM immediately; the normalization chain runs at
kc2-6 of the next position; AllToAll pieces trigger as soon as both
halves are normalized. All collective-gated work (recv DMAs + output
projections) is priority-pinned to the tail so launch/collective skew
can never stall the attention pipeline. ScalarE runs exp exclusively.
"""

import sys

if "/opt/trn_rl_repo" not in sys.path:
    sys.path.insert(0, "/opt/trn_rl_repo")

import numpy as np
import ml_dtypes

import concourse.bass as bass
import concourse.mybir as mybir
import concourse.tile as tile
from concourse import bacc
from concourse.bass import ts
from concourse.bass_utils import run_bass_kernel_spmd

BF16 = mybir.dt.bfloat16
F32 = mybir.dt.float32
ADD = mybir.AluOpType.add
MULT = mybir.AluOpType.mult
BYPASS = mybir.AluOpType.bypass
EXP = mybir.ActivationFunctionType.Exp

HEADS, D, H, N, B = 16, 64, 1024, 2048, 2
NC_ = 8
LH = 2            # local heads per core (one pair, both batches)
KC = 16           # k chunks of 128 over n=2048
QC = 4            # q chunks of 512 over n=2048 (= AllToAll pieces)
LVW = LH * 65     # 130: local v-aug width
LQK = LH * D      # 128 local q (or k) columns
GK, GQ = 0, 128   # wqk column groups (host-reordered): [K | Q]


def build_nc():
    nc = bacc.Bacc("TRN2", target_bir_lowering=False, debug=False, num_devices=NC_)

    # x for both batches, b-major columns: [h, b*N + pos]
    xT = nc.declare_dram_parameter("xT", [H, B * N], BF16, isOutput=False)
    wqk = nc.declare_dram_parameter("wqk", [H, 2 * LQK], BF16, isOutput=False)
    wv = nc.declare_dram_parameter("wv", [H, LVW], BF16, isOutput=False)
    wout = nc.declare_dram_parameter("wout", [H, H], BF16, isOutput=False)
    cos2 = nc.declare_dram_parameter("cos2", [64, N], BF16, isOutput=False)
    # sinm[p] = sin value read at SOURCE partition p during the shuffle:
    # p%64 < 32 -> +sin[p%64+32], else -sin[p%64-32]
    sinm = nc.declare_dram_parameter("sinm", [64, N], BF16, isOutput=False)
    out = nc.declare_dram_parameter("out", [QC, 128, H], BF16, isOutput=True)

    with tile.TileContext(nc) as tc:
        with (
            tc.tile_pool(name="dram", bufs=1, space="DRAM") as dram,
            tc.tile_pool(name="sb", bufs=1) as sb,
            tc.tile_pool(name="sbw", bufs=1) as sbw,
            tc.tile_pool(name="psum", bufs=2, space="PSUM") as ps,
        ):
            a2a_in = [dram.tile([8, 128, 128], BF16, name=f"ain{i}")
                      for i in range(QC)]
            a2a_out = [dram.tile([8, 128, 128], BF16, name=f"aout{i}")
                       for i in range(QC)]

            # warmup collective first: absorbs the one-time CC barrier
            # under the staging/projection prologue
            warm_in = dram.tile([8, 128], BF16, name="warm_in")
            warm_out = dram.tile([8, 128], BF16, name="warm_out")
            warm_sb = sbw.tile([1, 128], BF16)
            nc.vector.memset(warm_sb[:, :], 0.0)
            nc.scalar.dma_start(warm_in[0:1, :], warm_sb[:, :])
            nc.gpsimd.collective_compute(
                "AllToAll", BYPASS, replica_groups=[list(range(8))],
                ins=[warm_in.opt()], outs=[warm_out.opt()])

            # ---- staging: batch-0 x + K/Q groups + low cos/sin halves
            # gate the first exp; batch-1 x and wout trail ----
            xt_sb = sbw.tile([128, B * 8 * N], BF16)   # [p, b, hk, pos]
            wqk_sb = sbw.tile([128, 8 * 2 * LQK], BF16)
            wv_sb = sbw.tile([128, 8 * LVW], BF16)
            wout_sb = sbw.tile([128, 8 * H], BF16)
            cos2_sb = sbw.tile([128, N], BF16)
            sinm_sb = sbw.tile([128, N], BF16)
            ones_sb = sbw.tile([1, D], BF16)

            wqk_src = wqk.rearrange("(k p) c -> p k c", p=128)
            wqk_dst = wqk_sb.rearrange("p (k c) -> p k c", c=2 * LQK)

            def wqk_grp(eng, g):
                eng.dma_start(wqk_dst[:, :, g:g + 128],
                              wqk_src[:, :, g:g + 128])

            xt_src = xT.rearrange("(k p) (b c) -> p b k c", p=128, b=B)
            xt_dst = xt_sb.rearrange("p (b k c) -> p b k c", b=B, c=N)

            # x rides the SWDGE path (gpsimd): one big DMA splits across
            # all 16 SDMA engines while the HWDGE queues carry the small
            # weights in parallel
            nc.gpsimd.dma_start(xt_dst[:, 0], xt_src[:, 0])
            wqk_grp(nc.sync, GK)
            nc.sync.dma_start(cos2_sb[0:64, 0:1024], cos2[:, 0:1024])
            nc.sync.dma_start(sinm_sb[0:64, 0:1024], sinm[:, 0:1024])
            wqk_grp(nc.scalar, GQ)
            nc.scalar.dma_start(cos2_sb[0:64, 1024:2048], cos2[:, 1024:2048])
            nc.scalar.dma_start(sinm_sb[0:64, 1024:2048], sinm[:, 1024:2048])
            nc.scalar.dma_start(
                wv_sb.rearrange("p (k c) -> p k c", c=LVW),
                wv.rearrange("(k p) c -> p k c", p=128))
            # batch-1 x trails on the same SWDGE path
            nc.gpsimd.dma_start(xt_dst[:, 1], xt_src[:, 1])
            nc.vector.memset(ones_sb[:, :], 1.0)
            # duplicate cos/sin to partitions 64-127 (per-half, on DVE)
            for lo in (0, 1024):
                nc.vector.tensor_copy(cos2_sb[64:128, lo:lo + 1024],
                                      cos2_sb[0:64, lo:lo + 1024])
                nc.vector.tensor_copy(sinm_sb[64:128, lo:lo + 1024],
                                      sinm_sb[0:64, lo:lo + 1024])

            kt_rot = sb.tile([128, B * N], BF16)   # [batch b at b*N][n]
            qt_rot = sb.tile([128, B * N], BF16)
            vt_all = sb.tile([128, B * KC * LVW], BF16)
            # attn^T laid out [qc][b][row-block r][row-in-block]: shard
            # j = 4b+r of piece qc is the contiguous span (qc, b, r)
            attn_sb = sb.tile([128, B * N], BF16)
            attn4 = attn_sb.rearrange("p (q b r x) -> p q b r x", q=QC, b=B,
                                      x=128)

            # per-head ones columns of v-aug, set once
            nc.vector.memset(
                vt_all.rearrange("p (g e) -> p g e", e=65)[:, :, 64:65], 1.0)


            def proj_group(col0, b, sc):
                p = ps.tile([128, 512], F32, tag="b", name="pp")
                for hk in range(8):
                    nc.tensor.matmul(
                        p[:, :],
                        lhsT=wqk_sb[:, hk * 2 * LQK + col0:][:, :128],
                        rhs=xt_sb[:, (b * 8 + hk) * N + sc * 512:][:, :512],
                        start=(hk == 0),
                        stop=(hk == 7),
                    )
                return p

            def rotary_apply(psums, dst, pos0, W):
                """Rotary on a [128, W] span covering positions
                [pos0, pos0+W): stage psums to bf16 (DVE), then the
                partition-swapped sin multiply + cos multiply + add."""
                stage = sb.tile([128, 1024], BF16, tag="stg", bufs=2, name="stg")
                for i, p in enumerate(psums):
                    nc.vector.tensor_copy(stage[:, ts(i, 512)], p[:, :])
                tmp = sb.tile([128, 1024], BF16, tag="rta", bufs=2, name="rta")
                tmp2 = sb.tile([128, 1024], BF16, tag="rtb", bufs=2, name="rtb")
                sl = sinm_sb[:, pos0:pos0 + W]
                cl = cos2_sb[:, pos0:pos0 + W]
                for hh in (0, 64):
                    nc.vector.tensor_tensor(
                        tmp[hh:hh + 32, :W], stage[hh + 32:hh + 64, :W],
                        sl[hh + 32:hh + 64, :], MULT)
                    nc.vector.tensor_tensor(
                        tmp[hh + 32:hh + 64, :W], stage[hh:hh + 32, :W],
                        sl[hh:hh + 32, :], MULT)
                nc.vector.tensor_tensor(tmp2[:, :W], stage[:, :W], cl, MULT)
                nc.vector.tensor_tensor(dst, tmp2[:, :W], tmp[:, :W], ADD)

            def v_chunk(b, rc):
                p = ps.tile([128, LVW], F32, tag="b", name="vp")
                for hk in range(8):
                    nc.tensor.matmul(
                        p[:, :],
                        lhsT=xt_sb[:, (b * 8 + hk) * N + rc * 128:][:, :128],
                        rhs=wv_sb[:, ts(hk, LVW)],
                        start=(hk == 0),
                        stop=(hk == 7),
                    )
                nc.vector.tensor_copy(
                    vt_all[:, (b * KC + rc) * LVW:][:, :LVW].rearrange(
                        "p (h e) -> p h e", e=65)[:, :, 0:64],
                    p.rearrange("p (h e) -> p h e", e=65)[:, :, 0:64])

            def _av_mm(e, av0, av1, b, kc):
                base = (b * KC + kc) * LVW
                nc.tensor.matmul(
                    av0[:, :], lhsT=vt_all[:, base:][:, :65],
                    rhs=e[:, 0:512], start=(kc == 0), stop=(kc == KC - 1))
                nc.tensor.matmul(
                    av1[:, :], lhsT=vt_all[:, base + 65:][:, :65],
                    rhs=e[:, 512:1024], start=(kc == 0), stop=(kc == KC - 1))

            def finish_copy(av0, av1):
                # move attn_out^T (+denominator row 64) out of PSUM right
                # away so the next position's AV accumulation can reuse the
                # banks; also stage both denominator rows at partition 0 for
                # the broadcast matmul. Normalization itself runs later.
                a0 = sb.tile([65, 512], BF16, tag="avs", bufs=4, name="a0")
                a1 = sb.tile([65, 512], BF16, tag="avs", bufs=4, name="a1")
                nc.vector.tensor_copy(a0[:, :], av0[:, :])
                nc.vector.tensor_copy(a1[:, :], av1[:, :])
                ad = sb.tile([1, 1024], BF16, tag="adn", bufs=2, name="ad")
                nc.vector.tensor_copy(ad[:, 0:512], a0[64:65, :])
                nc.vector.tensor_copy(ad[:, 512:1024], a1[64:65, :])
                return a0, a1, ad

            def finish_norm(qc, b, a0, a1, ad):
                b_ps = ps.tile([128, 512], F32, tag="b", name="b_ps")
                nc.tensor.matmul(b_ps[0:64, :], lhsT=ones_sb[:, :],
                                 rhs=ad[:, 0:512], start=True, stop=True,
                                 tile_position=(0, 0))
                nc.tensor.matmul(b_ps[64:128, :], lhsT=ones_sb[:, :],
                                 rhs=ad[:, 512:1024], start=True, stop=True,
                                 tile_position=(0, 64))
                bd_sb = sb.tile([128, 512], F32, tag="bsd", bufs=2, name="bd_sb")
                nc.vector.tensor_copy(bd_sb[:, :], b_ps[:, :])
                b_sb = sb.tile([128, 512], F32, tag="bsb", bufs=2, name="b_sb")
                nc.vector.reciprocal_approx_fast(out=b_sb[:, :], in_=bd_sb[:, :])
                # both TT inputs must share a base partition: bring head B's
                # inv-denominators down to partitions 0-63
                b_lo = sb.tile([64, 512], F32, tag="blo", bufs=2, name="b_lo")
                nc.vector.tensor_copy(b_lo[:, :], b_sb[64:128, :])
                dst = attn4[:, qc, b, :, :]  # [128, 4, 128]
                b3 = b_sb.rearrange("p (j x) -> p j x", x=128)
                bl3 = b_lo.rearrange("p (j x) -> p j x", x=128)
                nc.vector.tensor_tensor(dst[0:64], a0[0:64, :].rearrange(
                    "p (j x) -> p j x", x=128), b3[0:64], MULT)
                nc.vector.tensor_tensor(dst[64:128], a1[0:64, :].rearrange(
                    "p (j x) -> p j x", x=128), bl3[:, :, :], MULT)

            def a2a_send(qc, b):
                # shard j=4b+r carries my 2 heads for (batch b, row block r)
                # of piece qc; b=None sends both batches in one DMA
                src = attn4[:, qc, :, :, :]          # [128, 2, 4, 128]
                d = a2a_in[qc].rearrange("(b r) p x -> p b r x", b=B)
                if b is not None:
                    src = src[:, b:b + 1, :, :]
                    d = d[:, b:b + 1, :, :]
                nc.sync.dma_start(d, src)

            def a2a_go(qc):
                nc.gpsimd.collective_compute(
                    "AllToAll", BYPASS, replica_groups=[list(range(8))],
                    ins=[a2a_in[qc].opt()], outs=[a2a_out[qc].opt()])

            def emit_a2a(qc):
                a2a_send(qc, None)
                a2a_go(qc)

            # tail-only: shard i of a2a_out = peer i's heads {2i, 2i+1} for
            # my 128 rows -> directly the outproj stationary operand
            def outproj_recv(qc):
                att_r = sb.tile([128, 8 * 128], BF16, tag="attr", bufs=2,
                                name="att_r")
                nc.gpsimd.dma_start(
                    att_r.rearrange("p (i x) -> p i x", i=8),
                    a2a_out[qc].rearrange("i p x -> p i x"))
                return att_r

            def outproj_mm(qc, att_r, nh):
                g3 = att_r.rearrange("p (c x) -> p c x", x=128)
                o_ps = ps.tile([128, 512], F32, tag="b", name="o_ps")
                for hc in range(8):
                    nc.tensor.matmul(
                        o_ps[:, :],
                        lhsT=g3[:, hc, :],
                        rhs=wout_sb[:, hc * H + nh * 512:][:, :512],
                        start=(hc == 0),
                        stop=(hc == 7),
                    )
                ob = sb.tile([128, 512], BF16, tag="ob", bufs=3, name="ob")
                nc.vector.tensor_copy(ob[:, :], o_ps[:, :])
                nc.sync.dma_start(out[qc, :, ts(nh, 512)], ob[:, :])

            def att_pos(qc, b, hooks):
                qt_p = qt_rot[:, b * N + qc * 512:][:, :512]
                av0 = ps.tile([65, 512], F32, tag="av", name="av0")
                av1 = ps.tile([65, 512], F32, tag="av", name="av1")
                exps = []
                for kc in range(KC):
                    s_ps = ps.tile([128, 1024], F32, tag="s", name="s_ps")
                    nc.tensor.matmul(
                        s_ps[:, 0:512],
                        lhsT=kt_rot[0:64, b * N + kc * 128:][:, :128],
                        rhs=qt_p[0:64, :], start=True, stop=True,
                        tile_position=(0, 0))
                    nc.tensor.matmul(
                        s_ps[:, 512:1024],
                        lhsT=kt_rot[64:128, b * N + kc * 128:][:, :128],
                        rhs=qt_p[64:128, :], start=True, stop=True,
                        tile_position=(64, 0))
                    e = sb.tile([128, 1024], BF16, tag="exp", bufs=6, name="e")
                    nc.scalar.activation(e[:, :], s_ps[:, :], EXP, scale=0.125)
                    exps.append(e)
                    for f in hooks.get(kc, []):
                        f()
                    if kc > 0:
                        _av_mm(exps[kc - 1], av0, av1, b, kc - 1)
                _av_mm(exps[KC - 1], av0, av1, b, KC - 1)
                return finish_copy(av0, av1)

            # ---- prologue ----
            # gate of the first exp: batch-0 K low half + Q sc0 + rotary;
            # V chunks 0-2 (b0) fill the PE while rotary runs on DVE
            k0a = [proj_group(GK, 0, 0), proj_group(GK, 0, 1)]
            rotary_apply(k0a, kt_rot[:, 0:1024], 0, 1024)
            q0s0 = [proj_group(GQ, 0, 0)]
            rotary_apply(q0s0, qt_rot[:, 0:512], 0, 512)
            for rc in range(3):
                v_chunk(0, rc)

            # closure helpers for hook tables
            def mk(f, *a):
                return lambda: f(*a)

            grabs = {}

            def grab(key, col0, b, sc):
                def g():
                    grabs.setdefault(key, []).append(proj_group(col0, b, sc))
                return g

            def rot(key, dst_tile, b, half):
                def g():
                    rotary_apply(
                        grabs.pop(key),
                        dst_tile[:, b * N + half * 1024:][:, :1024],
                        half * 1024, 1024)
                return g

            def rot_half(key, pos0):
                # 512-wide rotary chunk of batch-0 Q
                rotary_apply(grabs.pop(key),
                             qt_rot[:, pos0:pos0 + 512], pos0, 512)

            pend = {}

            def norm(qc, b):
                def g():
                    a0, a1, ad = pend.pop((qc, b))
                    finish_norm(qc, b, a0, a1, ad)
                return g

            def wout_dma():
                nc.sync.dma_start(
                    wout_sb.rearrange("p (k c) -> p k c", c=H),
                    wout.rearrange("(k p) c -> p k c", p=128))

            SEQ = [
                # (b, qc, hooks); norm(qc, b) finalizes the PREVIOUS
                # position; grab-pairs stay adjacent (2-buffer PSUM pool)
                (0, 0, dict(list({
                    0: [grab("k0b", GK, 0, 2)],
                    1: [grab("k0b", GK, 0, 3)],
                    2: [rot("k0b", kt_rot, 0, 1), mk(v_chunk, 0, 3)],
                    6: [grab("q0s1", GQ, 0, 1), mk(v_chunk, 0, 7)],
                    7: [mk(rot_half, "q0s1", 512), mk(v_chunk, 0, 8)],
                }.items()) + [(kc, [mk(v_chunk, 0, kc + 1)])
                              for kc in (3, 4, 5)]
                             + [(kc, [mk(v_chunk, 0, kc + 1)])
                                for kc in range(8, 15)])),
                (0, 1, {
                    1: [grab("k1a", GK, 1, 0)],
                    3: [grab("k1a", GK, 1, 1)],
                    5: [rot("k1a", kt_rot, 1, 0)],
                    6: [norm(0, 0), wout_dma],
                    7: [grab("q1a", GQ, 1, 0)],
                    9: [grab("q1a", GQ, 1, 1)],
                    11: [rot("q1a", qt_rot, 1, 0)],
                    12: [mk(v_chunk, 1, 0), mk(v_chunk, 1, 1),
                         mk(v_chunk, 1, 2)],
                    13: [mk(v_chunk, 1, 3), mk(v_chunk, 1, 4),
                         mk(v_chunk, 1, 5)],
                    14: [grab("k1b", GK, 1, 2)],
                    15: [grab("k1b", GK, 1, 3)],
                }),
                (1, 0, dict(list({
                    0: [rot("k1b", kt_rot, 1, 1)],
                    2: [norm(1, 0)],
                    3: [grab("q0b", GQ, 0, 2)],
                    4: [grab("q0b", GQ, 0, 3)],
                    5: [rot("q0b", qt_rot, 0, 1), mk(v_chunk, 1, 6)],
                }.items()) + [(kc, [mk(v_chunk, 1, kc + 1)])
                              for kc in range(6, 15)])),
                (0, 2, {
                    1: [grab("q1b", GQ, 1, 2)],
                    3: [grab("q1b", GQ, 1, 3)],
                    5: [rot("q1b", qt_rot, 1, 1)],
                    6: [norm(0, 1)],
                    8: [mk(emit_a2a, 0)],
                }),
                (1, 1, {
                    2: [norm(2, 0)],
                }),
                (0, 3, {
                    2: [norm(1, 1)],
                    6: [mk(emit_a2a, 1)],
                }),
                (1, 2, {
                    2: [norm(3, 0)],
                }),
                (1, 3, {
                    2: [norm(2, 1)],
                    4: [mk(emit_a2a, 2)],
                    8: [mk(a2a_send, 3, 0)],
                }),
            ]
            for b, qc, hooks in SEQ:
                pend[(qc, b)] = att_pos(qc, b, hooks)

            # tail: final normalization + piece-3 collective, then ALL
            # output projections (priority-pinned so the scheduler cannot
            # hoist collective-gated work into the attention stream)
            with tc.high_priority():
                norm(3, 1)()
                a2a_send(3, 1)
            a2a_go(3)
            with tc.high_priority(offset=-10_000_000):
                for qc in range(QC):
                    ar = outproj_recv(qc)
                    outproj_mm(qc, ar, 0)
                    outproj_mm(qc, ar, 1)

    nc.finalize()
    return nc


_NC = None


def _get_nc():
    global _NC
    if _NC is None:
        _NC = build_nc()
    return _NC


def _bf16(a):
    return np.ascontiguousarray(a.astype(ml_dtypes.bfloat16))


def make_in_maps(x, rotary_emb, w_qkv, w_out):
    x = np.asarray(x, np.float32)
    rotary_emb = np.asarray(rotary_emb, np.float32)
    w_qkv = np.asarray(w_qkv, np.float32)
    w_out = np.asarray(w_out, np.float32)
    cosT = np.cos(rotary_emb).T.astype(np.float32)  # [64, N]
    sinT = np.sin(rotary_emb).T.astype(np.float32)
    sswp = np.concatenate([sinT[32:], -sinT[:32]], axis=0)
    cos2_a = _bf16(cosT)
    sinm_a = _bf16(sswp)
    wout_bf = _bf16(w_out)
    xT_a = _bf16(np.concatenate([x[0].T, x[1].T], axis=1))  # [H, B*N]
    in_maps = []
    for c in range(NC_):
        h0 = LH * c  # heads {2c, 2c+1}
        wq_loc = w_qkv[:, 64 * h0: 64 * h0 + LQK]
        wk_loc = w_qkv[:, H + 64 * h0: H + 64 * h0 + LQK]
        wv_loc = w_qkv[:, 2 * H + 64 * h0: 2 * H + 64 * h0 + LQK]
        wv_aug = np.zeros((H, LVW), np.float32)
        for j in range(LH):
            wv_aug[:, 65 * j: 65 * j + 64] = wv_loc[:, 64 * j: 64 * j + 64]
        # column-grouped wqk: [K | Q]
        wqk_g = np.concatenate([wk_loc, wq_loc], axis=1)
        in_maps.append({
            "xT": xT_a,
            "wqk": _bf16(wqk_g),
            "wv": _bf16(wv_aug),
            "wout": wout_bf,
            "cos2": cos2_a,
            "sinm": sinm_a,
        })
    return in_maps


def run(x, rotary_emb, w_qkv, w_out, trace=False, tmpdir=None):
    nc = _get_nc()
    in_maps = make_in_maps(x, rotary_emb, w_qkv, w_out)
    res = run_bass_kernel_spmd(nc, in_maps, list(range(NC_)), trace=trace,
                               tmpdir=tmpdir)
    full = np.empty((B, N, H), np.float32)
    for c in range(NC_):
        b, r = c // 4, c % 4
        piece = np.asarray(res.results[c]["out"]).astype(np.float32)
        for qc in range(QC):
            full[b, 512 * qc + 128 * r: 512 * qc + 128 * r + 128] = piece[qc]
    return full, res


def kernel(x, rotary_emb, w_qkv, w_out):
    full, _ = run(x, rotary_emb, w_qkv, w_out)
    return full


# revision 41
# speedup vs baseline: 1.0496x; 1.0394x over previous
"""Trainium2 Bass kernel for fused multi-head attention (16 heads, d=64,
b=2, n=2048, h=1024) across 8 NeuronCores.

Sharding: 2 heads x BOTH batches per core (core c owns heads {2c, 2c+1}).
With every core holding a slice of every batch, the post-attention
Ulysses AllToAll carries no batch duplication: each 512-row piece is a
256KB exchange whose every shard is useful, and the received shards feed
the output projection directly (no mask-combine). Each core then runs
the output projection for its 128-row slice of batch c//4 over the full
1024-dim contraction.

Schedule: one software-pipelined stream, TensorE-throughput-bound at the
power-capped PE clock (~263ns per 512-wide matmul). Staging is arranged
so only batch-0 x + the K/Q weight column groups + the low cos/sin
halves gate the first exp; batch-1 x and the output weights trail behind.
Attention positions run (b,qc) = (0,0),(0,1),(1,0),(0,2),(1,1),(0,3),
(1,2),(1,3); projections and V chunks are interleaved just-in-time into
the attention matmul stream (grab 512-wide psums, then rotate; pairs
kept adjacent so the 2-buffer PSUM pool never head-of-line blocks the PE
queue). Each position's attn_out^T (+denominator row from a ones-column
in V) is copied out of PSUM immediately; the normalization chain runs at
kc2-6 of the next position; AllToAll pieces trigger as soon as both
halves are normalized. All collective-gated work (recv DMAs + output
projections) is priority-pinned to the tail so launch/collective skew
can never stall the attention pipeline. ScalarE runs exp exclusively.
"""

import sys

if "/opt/trn_rl_repo" not in sys.path:
    sys.path.insert(0, "/opt/trn_rl_repo")

import numpy as np
import ml_dtypes

import concourse.bass as bass
import concourse.mybir as mybir
import concourse.tile as tile
from concourse import bacc
from concourse.bass import ts
from concourse.bass_utils import run_bass_kernel_spmd

BF16 = mybir.dt.bfloat16
F32 = mybir.dt.float32
ADD = mybir.AluOpType.add
MULT = mybir.AluOpType.mult
BYPASS = mybir.AluOpType.bypass
EXP = mybir.ActivationFunctionType.Exp

HEADS, D, H, N, B = 16, 64, 1024, 2048, 2
NC_ = 8
LH = 2            # local heads per core (one pair, both batches)
KC = 16           # k chunks of 128 over n=2048
QC = 4            # q chunks of 512 over n=2048 (= AllToAll pieces)
LVW = LH * 65     # 130: local v-aug width
LQK = LH * D      # 128 local q (or k) columns
GK, GQ = 0, 128   # wqk column groups (host-reordered): [K | Q]


def build_nc():
    nc = bacc.Bacc("TRN2", target_bir_lowering=False, debug=False, num_devices=NC_)

    # x for both batches, b-major columns: [h, b*N + pos]
    xT = nc.declare_dram_parameter("xT", [H, B * N], BF16, isOutput=False)
    wqk = nc.declare_dram_parameter("wqk", [H, 2 * LQK], BF16, isOutput=False)
    wv = nc.declare_dram_parameter("wv", [H, LVW], BF16, isOutput=False)
    wout = nc.declare_dram_parameter("wout", [H, H], BF16, isOutput=False)
    cos2 = nc.declare_dram_parameter("cos2", [64, N], BF16, isOutput=False)
    # sinm[p] = sin value read at SOURCE partition p during the shuffle:
    # p%64 < 32 -> +sin[p%64+32], else -sin[p%64-32]
    sinm = nc.declare_dram_parameter("sinm", [64, N], BF16, isOutput=False)
    out = nc.declare_dram_parameter("out", [QC, 128, H], BF16, isOutput=True)

    with tile.TileContext(nc) as tc:
        with (
            tc.tile_pool(name="dram", bufs=1, space="DRAM") as dram,
            tc.tile_pool(name="sb", bufs=1) as sb,
            tc.tile_pool(name="sbw", bufs=1) as sbw,
            tc.tile_pool(name="psum", bufs=2, space="PSUM") as ps,
        ):
            a2a_in = [dram.tile([8, 128, 128], BF16, name=f"ain{i}")
                      for i in range(QC)]
            a2a_out = [dram.tile([8, 128, 128], BF16, name=f"aout{i}")
                       for i in range(QC)]

            # warmup collective first: absorbs the one-time CC barrier
            # under the staging/projection prologue
            warm_in = dram.tile([8, 128], BF16, name="warm_in")
            warm_out = dram.tile([8, 128], BF16, name="warm_out")
            warm_sb = sbw.tile([1, 128], BF16)
            nc.vector.memset(warm_sb[:, :], 0.0)
            nc.scalar.dma_start(warm_in[0:1, :], warm_sb[:, :])
            nc.gpsimd.collective_compute(
                "AllToAll", BYPASS, replica_groups=[list(range(8))],
                ins=[warm_in.opt()], outs=[warm_out.opt()])

            # ---- staging: batch-0 x + K/Q groups + low cos/sin halves
            # gate the first exp; batch-1 x and wout trail ----
            xt_sb = sbw.tile([128, B * 8 * N], BF16)   # [p, b, hk, pos]
            wqk_sb = sbw.tile([128, 8 * 2 * LQK], BF16)
            wv_sb = sbw.tile([128, 8 * LVW], BF16)
            wout_sb = sbw.tile([128, 8 * H], BF16)
            cos2_sb = sbw.tile([128, N], BF16)
            sinm_sb = sbw.tile([128, N], BF16)
            ones_sb = sbw.tile([1, D], BF16)

            wqk_src = wqk.rearrange("(k p) c -> p k c", p=128)
            wqk_dst = wqk_sb.rearrange("p (k c) -> p k c", c=2 * LQK)

            def wqk_grp(eng, g):
                eng.dma_start(wqk_dst[:, :, g:g + 128],
                              wqk_src[:, :, g:g + 128])

            xt_src = xT.rearrange("(k p) (b c) -> p b k c", p=128, b=B)
            xt_dst = xt_sb.rearrange("p (b k c) -> p b k c", b=B, c=N)

            # x rides the SWDGE path (gpsimd): one big DMA splits across
            # all 16 SDMA engines while the HWDGE queues carry the small
            # weights in parallel
            nc.gpsimd.dma_start(xt_dst[:, 0], xt_src[:, 0])
            wqk_grp(nc.sync, GK)
            nc.sync.dma_start(cos2_sb[0:64, 0:1024], cos2[:, 0:1024])
            nc.sync.dma_start(sinm_sb[0:64, 0:1024], sinm[:, 0:1024])
            wqk_grp(nc.scalar, GQ)
            nc.scalar.dma_start(cos2_sb[0:64, 1024:2048], cos2[:, 1024:2048])
            nc.scalar.dma_start(sinm_sb[0:64, 1024:2048], sinm[:, 1024:2048])
            nc.scalar.dma_start(
                wv_sb.rearrange("p (k c) -> p k c", c=LVW),
                wv.rearrange("(k p) c -> p k c", p=128))
            # batch-1 x trails on the same SWDGE path
            nc.gpsimd.dma_start(xt_dst[:, 1], xt_src[:, 1])
            nc.vector.memset(ones_sb[:, :], 1.0)
            # duplicate cos/sin to partitions 64-127 (per-half, on DVE)
            for lo in (0, 1024):
                nc.vector.tensor_copy(cos2_sb[64:128, lo:lo + 1024],
                                      cos2_sb[0:64, lo:lo + 1024])
                nc.vector.tensor_copy(sinm_sb[64:128, lo:lo + 1024],
                                      sinm_sb[0:64, lo:lo + 1024])

            kt_rot = sb.tile([128, B * N], BF16)   # [batch b at b*N][n]
            qt_rot = sb.tile([128, B * N], BF16)
            vt_all = sb.tile([128, B * KC * LVW], BF16)
            # attn^T laid out [qc][b][row-block r][row-in-block]: shard
            # j = 4b+r of piece qc is the contiguous span (qc, b, r)
            attn_sb = sb.tile([128, B * N], BF16)
            attn4 = attn_sb.rearrange("p (q b r x) -> p q b r x", q=QC, b=B,
                                      x=128)

            # per-head ones columns of v-aug, set once
            nc.vector.memset(
                vt_all.rearrange("p (g e) -> p g e", e=65)[:, :, 64:65], 1.0)


            def proj_group(col0, b, sc):
                p = ps.tile([128, 512], F32, tag="b", name="pp")
                for hk in range(8):
                    nc.tensor.matmul(
                        p[:, :],
                        lhsT=wqk_sb[:, hk * 2 * LQK + col0:][:, :128],
                        rhs=xt_sb[:, (b * 8 + hk) * N + sc * 512:][:, :512],
                        start=(hk == 0),
                        stop=(hk == 7),
                    )
                return p

            def rotary_apply(psums, dst, pos0, W):
                """Rotary on a [128, W] span covering positions
                [pos0, pos0+W): stage psums to bf16 (DVE), then the
                partition-swapped sin multiply + cos multiply + add."""
                stage = sb.tile([128, 1024], BF16, tag="stg", bufs=2, name="stg")
                for i, p in enumerate(psums):
                    nc.vector.tensor_copy(stage[:, ts(i, 512)], p[:, :])
                tmp = sb.tile([128, 1024], BF16, tag="rta", bufs=2, name="rta")
                tmp2 = sb.tile([128, 1024], BF16, tag="rtb", bufs=2, name="rtb")
                sl = sinm_sb[:, pos0:pos0 + W]
                cl = cos2_sb[:, pos0:pos0 + W]
                for hh in (0, 64):
                    nc.vector.tensor_tensor(
                        tmp[hh:hh + 32, :W], stage[hh + 32:hh + 64, :W],
                        sl[hh + 32:hh + 64, :], MULT)
                    nc.vector.tensor_tensor(
                        tmp[hh + 32:hh + 64, :W], stage[hh:hh + 32, :W],
                        sl[hh:hh + 32, :], MULT)
                nc.vector.tensor_tensor(tmp2[:, :W], stage[:, :W], cl, MULT)
                nc.vector.tensor_tensor(dst, tmp2[:, :W], tmp[:, :W], ADD)

            def v_chunk(b, rc):
                p = ps.tile([128, LVW], F32, tag="b", name="vp")
                for hk in range(8):
                    nc.tensor.matmul(
                        p[:, :],
                        lhsT=xt_sb[:, (b * 8 + hk) * N + rc * 128:][:, :128],
                        rhs=wv_sb[:, ts(hk, LVW)],
                        start=(hk == 0),
                        stop=(hk == 7),
                    )
                nc.vector.tensor_copy(
                    vt_all[:, (b * KC + rc) * LVW:][:, :LVW].rearrange(
                        "p (h e) -> p h e", e=65)[:, :, 0:64],
                    p.rearrange("p (h e) -> p h e", e=65)[:, :, 0:64])

            def _av_mm(e, av0, av1, b, kc):
                base = (b * KC + kc) * LVW
                nc.tensor.matmul(
                    av0[:, :], lhsT=vt_all[:, base:][:, :65],
                    rhs=e[:, 0:512], start=(kc == 0), stop=(kc == KC - 1))
                nc.tensor.matmul(
                    av1[:, :], lhsT=vt_all[:, base + 65:][:, :65],
                    rhs=e[:, 512:1024], start=(kc == 0), stop=(kc == KC - 1))

            def finish_copy(av0, av1):
                # move attn_out^T (+denominator row 64) out of PSUM right
                # away so the next position's AV accumulation can reuse the
                # banks; also stage both denominator rows at partition 0 for
                # the broadcast matmul. Normalization itself runs later.
                a0 = sb.tile([65, 512], BF16, tag="avs", bufs=4, name="a0")
                a1 = sb.tile([65, 512], BF16, tag="avs", bufs=4, name="a1")
                nc.vector.tensor_copy(a0[:, :], av0[:, :])
                nc.vector.tensor_copy(a1[:, :], av1[:, :])
                ad = sb.tile([1, 1024], BF16, tag="adn", bufs=2, name="ad")
                nc.vector.tensor_copy(ad[:, 0:512], a0[64:65, :])
                nc.vector.tensor_copy(ad[:, 512:1024], a1[64:65, :])
                return a0, a1, ad

            def finish_norm(qc, b, a0, a1, ad):
                b_ps = ps.tile([128, 512], F32, tag="b", name="b_ps")
                nc.tensor.matmul(b_ps[0:64, :], lhsT=ones_sb[:, :],
                                 rhs=ad[:, 0:512], start=True, stop=True,
                                 tile_position=(0, 0))
                nc.tensor.matmul(b_ps[64:128, :], lhsT=ones_sb[:, :],
                                 rhs=ad[:, 512:1024], start=True, stop=True,
                                 tile_position=(0, 64))
                bd_sb = sb.tile([128, 512], F32, tag="bsd", bufs=2, name="bd_sb")
                nc.vector.tensor_copy(bd_sb[:, :], b_ps[:, :])
                b_sb = sb.tile([128, 512], F32, tag="bsb", bufs=2, name="b_sb")
                nc.vector.reciprocal_approx_fast(out=b_sb[:, :], in_=bd_sb[:, :])
                # both TT inputs must share a base partition: bring head B's
                # inv-denominators down to partitions 0-63
                b_lo = sb.tile([64, 512], F32, tag="blo", bufs=2, name="b_lo")
                nc.vector.tensor_copy(b_lo[:, :], b_sb[64:128, :])
                dst = attn4[:, qc, b, :, :]  # [128, 4, 128]
                b3 = b_sb.rearrange("p (j x) -> p j x", x=128)
                bl3 = b_lo.rearrange("p (j x) -> p j x", x=128)
                nc.vector.tensor_tensor(dst[0:64], a0[0:64, :].rearrange(
                    "p (j x) -> p j x", x=128), b3[0:64], MULT)
                nc.vector.tensor_tensor(dst[64:128], a1[0:64, :].rearrange(
                    "p (j x) -> p j x", x=128), bl3[:, :, :], MULT)

            def a2a_send(qc, b):
                # shard j=4b+r carries my 2 heads for (batch b, row block r)
                # of piece qc; b=None sends both batches in one DMA
                src = attn4[:, qc, :, :, :]          # [128, 2, 4, 128]
                d = a2a_in[qc].rearrange("(b r) p x -> p b r x", b=B)
                if b is not None:
                    src = src[:, b:b + 1, :, :]
                    d = d[:, b:b + 1, :, :]
                nc.sync.dma_start(d, src)

            def a2a_go(qc):
                nc.gpsimd.collective_compute(
                    "AllToAll", BYPASS, replica_groups=[list(range(8))],
                    ins=[a2a_in[qc].opt()], outs=[a2a_out[qc].opt()])

            def emit_a2a(qc):
                a2a_send(qc, None)
                a2a_go(qc)

            # tail-only: shard i of a2a_out = peer i's heads {2i, 2i+1} for
            # my 128 rows -> directly the outproj stationary operand
            def outproj_recv(qc):
                att_r = sb.tile([128, 8 * 128], BF16, tag="attr", bufs=2,
                                name="att_r")
                nc.gpsimd.dma_start(
                    att_r.rearrange("p (i x) -> p i x", i=8),
                    a2a_out[qc].rearrange("i p x -> p i x"))
                return att_r

            def outproj_mm(qc, att_r, nh):
                g3 = att_r.rearrange("p (c x) -> p c x", x=128)
                o_ps = ps.tile([128, 512], F32, tag="b", name="o_ps")
                for hc in range(8):
                    nc.tensor.matmul(
                        o_ps[:, :],
                        lhsT=g3[:, hc, :],
                        rhs=wout_sb[:, hc * H + nh * 512:][:, :512],
                        start=(hc == 0),
                        stop=(hc == 7),
                    )
                ob = sb.tile([128, 512], BF16, tag="ob", bufs=3, name="ob")
                nc.vector.tensor_copy(ob[:, :], o_ps[:, :])
                nc.sync.dma_start(out[qc, :, ts(nh, 512)], ob[:, :])

            def att_pos(qc, b, hooks):
                qt_p = qt_rot[:, b * N + qc * 512:][:, :512]
                av0 = ps.tile([65, 512], F32, tag="av", name="av0")
                av1 = ps.tile([65, 512], F32, tag="av", name="av1")
                exps = []
                for kc in range(KC):
                    s_ps = ps.tile([128, 1024], F32, tag="s", name="s_ps")
                    nc.tensor.matmul(
                        s_ps[:, 0:512],
                        lhsT=kt_rot[0:64, b * N + kc * 128:][:, :128],
                        rhs=qt_p[0:64, :], start=True, stop=True,
                        tile_position=(0, 0))
                    nc.tensor.matmul(
                        s_ps[:, 512:1024],
                        lhsT=kt_rot[64:128, b * N + kc * 128:][:, :128],
                        rhs=qt_p[64:128, :], start=True, stop=True,
                        tile_position=(64, 0))
                    e = sb.tile([128, 1024], BF16, tag="exp", bufs=8, name="e")
                    nc.scalar.activation(e[:, :], s_ps[:, :], EXP, scale=0.125)
                    exps.append(e)
                    for f in hooks.get(kc, []):
                        f()
                    if kc > 0:
                        _av_mm(exps[kc - 1], av0, av1, b, kc - 1)
                _av_mm(exps[KC - 1], av0, av1, b, KC - 1)
                return finish_copy(av0, av1)

            # ---- prologue ----
            # gate of the first exp: batch-0 K low half + Q sc0 + rotary;
            # V chunks 0-2 (b0) fill the PE while rotary runs on DVE
            k0a = [proj_group(GK, 0, 0), proj_group(GK, 0, 1)]
            rotary_apply(k0a, kt_rot[:, 0:1024], 0, 1024)
            q0s0 = [proj_group(GQ, 0, 0)]
            rotary_apply(q0s0, qt_rot[:, 0:512], 0, 512)
            for rc in range(3):
                v_chunk(0, rc)

            # closure helpers for hook tables
            def mk(f, *a):
                return lambda: f(*a)

            grabs = {}

            def grab(key, col0, b, sc):
                def g():
                    grabs.setdefault(key, []).append(proj_group(col0, b, sc))
                return g

            def rot(key, dst_tile, b, half):
                def g():
                    rotary_apply(
                        grabs.pop(key),
                        dst_tile[:, b * N + half * 1024:][:, :1024],
                        half * 1024, 1024)
                return g

            def rot_half(key, pos0):
                # 512-wide rotary chunk of batch-0 Q
                rotary_apply(grabs.pop(key),
                             qt_rot[:, pos0:pos0 + 512], pos0, 512)

            pend = {}

            def norm(qc, b):
                def g():
                    a0, a1, ad = pend.pop((qc, b))
                    finish_norm(qc, b, a0, a1, ad)
                return g

            def wout_dma():
                nc.sync.dma_start(
                    wout_sb.rearrange("p (k c) -> p k c", c=H),
                    wout.rearrange("(k p) c -> p k c", p=128))

            SEQ = [
                # (b, qc, hooks); norm(qc, b) finalizes the PREVIOUS
                # position; grab-pairs stay adjacent (2-buffer PSUM pool)
                (0, 0, dict(list({
                    0: [grab("k0b", GK, 0, 2)],
                    1: [grab("k0b", GK, 0, 3)],
                    2: [rot("k0b", kt_rot, 0, 1), mk(v_chunk, 0, 3)],
                    6: [grab("q0s1", GQ, 0, 1), mk(v_chunk, 0, 7)],
                    7: [mk(rot_half, "q0s1", 512), mk(v_chunk, 0, 8)],
                }.items()) + [(kc, [mk(v_chunk, 0, kc + 1)])
                              for kc in (3, 4, 5)]
                             + [(kc, [mk(v_chunk, 0, kc + 1)])
                                for kc in range(8, 15)])),
                (0, 1, {
                    1: [grab("k1a", GK, 1, 0)],
                    3: [grab("k1a", GK, 1, 1)],
                    5: [rot("k1a", kt_rot, 1, 0)],
                    6: [norm(0, 0), wout_dma],
                    7: [grab("q1a", GQ, 1, 0)],
                    9: [grab("q1a", GQ, 1, 1)],
                    11: [rot("q1a", qt_rot, 1, 0)],
                    12: [mk(v_chunk, 1, 0), mk(v_chunk, 1, 1),
                         mk(v_chunk, 1, 2)],
                    13: [mk(v_chunk, 1, 3), mk(v_chunk, 1, 4),
                         mk(v_chunk, 1, 5)],
                    14: [grab("k1b", GK, 1, 2)],
                    15: [grab("k1b", GK, 1, 3)],
                }),
                (1, 0, dict(list({
                    0: [rot("k1b", kt_rot, 1, 1)],
                    2: [norm(1, 0)],
                    3: [grab("q0b", GQ, 0, 2)],
                    4: [grab("q0b", GQ, 0, 3)],
                    5: [rot("q0b", qt_rot, 0, 1), mk(v_chunk, 1, 6)],
                }.items()) + [(kc, [mk(v_chunk, 1, kc + 1)])
                              for kc in range(6, 15)])),
                (0, 2, {
                    1: [grab("q1b", GQ, 1, 2)],
                    3: [grab("q1b", GQ, 1, 3)],
                    5: [rot("q1b", qt_rot, 1, 1)],
                    6: [norm(0, 1)],
                    8: [mk(emit_a2a, 0)],
                }),
                (1, 1, {
                    2: [norm(2, 0)],
                }),
                (0, 3, {
                    2: [norm(1, 1)],
                    6: [mk(emit_a2a, 1)],
                }),
                (1, 2, {
                    2: [norm(3, 0)],
                }),
                (1, 3, {
                    2: [norm(2, 1)],
                    4: [mk(emit_a2a, 2)],
                    8: [mk(a2a_send, 3, 0)],
                }),
            ]
            for b, qc, hooks in SEQ:
                pend[(qc, b)] = att_pos(qc, b, hooks)

            # tail: final normalization + piece-3 collective, then ALL
            # output projections (priority-pinned so the scheduler cannot
            # hoist collective-gated work into the attention stream)
            with tc.high_priority():
                norm(3, 1)()
                a2a_send(3, 1)
            a2a_go(3)
            with tc.high_priority(offset=-10_000_000):
                for qc in range(QC):
                    ar = outproj_recv(qc)
                    outproj_mm(qc, ar, 0)
                    outproj_mm(qc, ar, 1)

    nc.finalize()
    return nc


_NC = None


def _get_nc():
    global _NC
    if _NC is None:
        _NC = build_nc()
    return _NC


def _bf16(a):
    return np.ascontiguousarray(a.astype(ml_dtypes.bfloat16))


def make_in_maps(x, rotary_emb, w_qkv, w_out):
    x = np.asarray(x, np.float32)
    rotary_emb = np.asarray(rotary_emb, np.float32)
    w_qkv = np.asarray(w_qkv, np.float32)
    w_out = np.asarray(w_out, np.float32)
    cosT = np.cos(rotary_emb).T.astype(np.float32)  # [64, N]
    sinT = np.sin(rotary_emb).T.astype(np.float32)
    sswp = np.concatenate([sinT[32:], -sinT[:32]], axis=0)
    cos2_a = _bf16(cosT)
    sinm_a = _bf16(sswp)
    wout_bf = _bf16(w_out)
    xT_a = _bf16(np.concatenate([x[0].T, x[1].T], axis=1))  # [H, B*N]
    in_maps = []
    for c in range(NC_):
        h0 = LH * c  # heads {2c, 2c+1}
        wq_loc = w_qkv[:, 64 * h0: 64 * h0 + LQK]
        wk_loc = w_qkv[:, H + 64 * h0: H + 64 * h0 + LQK]
        wv_loc = w_qkv[:, 2 * H + 64 * h0: 2 * H + 64 * h0 + LQK]
        wv_aug = np.zeros((H, LVW), np.float32)
        for j in range(LH):
            wv_aug[:, 65 * j: 65 * j + 64] = wv_loc[:, 64 * j: 64 * j + 64]
        # column-grouped wqk: [K | Q]
        wqk_g = np.concatenate([wk_loc, wq_loc], axis=1)
        in_maps.append({
            "xT": xT_a,
            "wqk": _bf16(wqk_g),
            "wv": _bf16(wv_aug),
            "wout": wout_bf,
            "cos2": cos2_a,
            "sinm": sinm_a,
        })
    return in_maps


def run(x, rotary_emb, w_qkv, w_out, trace=False, tmpdir=None):
    nc = _get_nc()
    in_maps = make_in_maps(x, rotary_emb, w_qkv, w_out)
    res = run_bass_kernel_spmd(nc, in_maps, list(range(NC_)), trace=trace,
                               tmpdir=tmpdir)
    full = np.empty((B, N, H), np.float32)
    for c in range(NC_):
        b, r = c // 4, c % 4
        piece = np.asarray(res.results[c]["out"]).astype(np.float32)
        for qc in range(QC):
            full[b, 512 * qc + 128 * r: 512 * qc + 128 * r + 128] = piece[qc]
    return full, res


def kernel(x, rotary_emb, w_qkv, w_out):
    full, _ = run(x, rotary_emb, w_qkv, w_out)
    return full


# revision 42
# speedup vs baseline: 1.1169x; 1.0641x over previous
"""Trainium2 Bass kernel for fused multi-head attention (16 heads, d=64,
b=2, n=2048, h=1024) across 8 NeuronCores.

Sharding: 2 heads x BOTH batches per core (core c owns heads {2c, 2c+1}).
With every core holding a slice of every batch, the post-attention
Ulysses AllToAll carries no batch duplication: each 512-row piece is a
256KB exchange whose every shard is useful, and the received shards feed
the output projection directly (no mask-combine). Each core then runs
the output projection for its 128-row slice of batch c//4 over the full
1024-dim contraction.

Schedule: one software-pipelined stream, TensorE-throughput-bound at the
power-capped PE clock (~263ns per 512-wide matmul). Staging is arranged
so only batch-0 x + the K/Q weight column groups + the low cos/sin
halves gate the first exp; batch-1 x and the output weights trail behind.
Attention positions run (b,qc) = (0,0),(0,1),(1,0),(0,2),(1,1),(0,3),
(1,2),(1,3); projections and V chunks are interleaved just-in-time into
the attention matmul stream (grab 512-wide psums, then rotate; pairs
kept adjacent so the 2-buffer PSUM pool never head-of-line blocks the PE
queue). Each position's attn_out^T (+denominator row from a ones-column
in V) is copied out of PSUM immediately; the normalization chain runs at
kc2-6 of the next position; AllToAll pieces trigger as soon as both
halves are normalized. All collective-gated work (recv DMAs + output
projections) is priority-pinned to the tail so launch/collective skew
can never stall the attention pipeline. ScalarE runs exp exclusively.
"""

import sys

if "/opt/trn_rl_repo" not in sys.path:
    sys.path.insert(0, "/opt/trn_rl_repo")

import numpy as np
import ml_dtypes

import concourse.bass as bass
import concourse.mybir as mybir
import concourse.tile as tile
from concourse import bacc
from concourse.bass import ts
from concourse.bass_utils import run_bass_kernel_spmd

BF16 = mybir.dt.bfloat16
F32 = mybir.dt.float32
ADD = mybir.AluOpType.add
MULT = mybir.AluOpType.mult
BYPASS = mybir.AluOpType.bypass
EXP = mybir.ActivationFunctionType.Exp

HEADS, D, H, N, B = 16, 64, 1024, 2048, 2
NC_ = 8
LH = 2            # local heads per core (one pair, both batches)
KC = 16           # k chunks of 128 over n=2048
QC = 4            # q chunks of 512 over n=2048 (= AllToAll pieces)
LVW = LH * 65     # 130: local v-aug width
LQK = LH * D      # 128 local q (or k) columns
GK, GQ = 0, 128   # wqk column groups (host-reordered): [K | Q]


def build_nc():
    nc = bacc.Bacc("TRN2", target_bir_lowering=False, debug=False, num_devices=NC_)

    # x for both batches, b-major columns: [h, b*N + pos]
    xT = nc.declare_dram_parameter("xT", [H, B * N], BF16, isOutput=False)
    wqk = nc.declare_dram_parameter("wqk", [H, 2 * LQK], BF16, isOutput=False)
    wv = nc.declare_dram_parameter("wv", [H, LVW], BF16, isOutput=False)
    wout = nc.declare_dram_parameter("wout", [H, H], BF16, isOutput=False)
    cos2 = nc.declare_dram_parameter("cos2", [64, N], BF16, isOutput=False)
    # sinm[p] = sin value read at SOURCE partition p during the shuffle:
    # p%64 < 32 -> +sin[p%64+32], else -sin[p%64-32]
    sinm = nc.declare_dram_parameter("sinm", [64, N], BF16, isOutput=False)
    out = nc.declare_dram_parameter("out", [QC, 128, H], BF16, isOutput=True)

    with tile.TileContext(nc) as tc:
        with (
            tc.tile_pool(name="dram", bufs=1, space="DRAM") as dram,
            tc.tile_pool(name="sb", bufs=1) as sb,
            tc.tile_pool(name="sbw", bufs=1) as sbw,
            tc.tile_pool(name="psum", bufs=2, space="PSUM") as ps,
        ):
            a2a_in = [dram.tile([8, 128, 128], BF16, name=f"ain{i}")
                      for i in range(QC)]
            a2a_out = [dram.tile([8, 128, 128], BF16, name=f"aout{i}")
                       for i in range(QC)]

            # warmup collective first: absorbs the one-time CC barrier
            # under the staging/projection prologue
            warm_in = dram.tile([8, 128], BF16, name="warm_in")
            warm_out = dram.tile([8, 128], BF16, name="warm_out")
            warm_sb = sbw.tile([1, 128], BF16)
            nc.vector.memset(warm_sb[:, :], 0.0)
            nc.scalar.dma_start(warm_in[0:1, :], warm_sb[:, :])
            nc.gpsimd.collective_compute(
                "AllToAll", BYPASS, replica_groups=[list(range(8))],
                ins=[warm_in.opt()], outs=[warm_out.opt()])

            # ---- staging: batch-0 x + K/Q groups + low cos/sin halves
            # gate the first exp; batch-1 x and wout trail ----
            xt_sb = sbw.tile([128, B * 8 * N], BF16)   # [p, b, hk, pos]
            wqk_sb = sbw.tile([128, 8 * 2 * LQK], BF16)
            wv_sb = sbw.tile([128, 8 * LVW], BF16)
            wout_sb = sbw.tile([128, 8 * H], BF16)
            cos2_sb = sbw.tile([128, N], BF16)
            sinm_sb = sbw.tile([128, N], BF16)
            ones_sb = sbw.tile([1, D], BF16)

            wqk_src = wqk.rearrange("(k p) c -> p k c", p=128)
            wqk_dst = wqk_sb.rearrange("p (k c) -> p k c", c=2 * LQK)

            def wqk_grp(eng, g):
                eng.dma_start(wqk_dst[:, :, g:g + 128],
                              wqk_src[:, :, g:g + 128])

            xt_src = xT.rearrange("(k p) (b c) -> p b k c", p=128, b=B)
            xt_dst = xt_sb.rearrange("p (b k c) -> p b k c", b=B, c=N)

            # x rides the SWDGE path (gpsimd): one big DMA splits across
            # all 16 SDMA engines while the HWDGE queues carry the small
            # weights in parallel
            nc.gpsimd.dma_start(xt_dst[:, 0], xt_src[:, 0])
            wqk_grp(nc.sync, GK)
            nc.sync.dma_start(cos2_sb[0:64, 0:1024], cos2[:, 0:1024])
            nc.sync.dma_start(sinm_sb[0:64, 0:1024], sinm[:, 0:1024])
            wqk_grp(nc.scalar, GQ)
            nc.scalar.dma_start(cos2_sb[0:64, 1024:2048], cos2[:, 1024:2048])
            nc.scalar.dma_start(sinm_sb[0:64, 1024:2048], sinm[:, 1024:2048])
            nc.scalar.dma_start(
                wv_sb.rearrange("p (k c) -> p k c", c=LVW),
                wv.rearrange("(k p) c -> p k c", p=128))
            # batch-1 x trails on the same SWDGE path
            nc.gpsimd.dma_start(xt_dst[:, 1], xt_src[:, 1])
            nc.vector.memset(ones_sb[:, :], 1.0)
            # duplicate cos/sin to partitions 64-127 (per-half, on DVE)
            for lo in (0, 1024):
                nc.vector.tensor_copy(cos2_sb[64:128, lo:lo + 1024],
                                      cos2_sb[0:64, lo:lo + 1024])
                nc.vector.tensor_copy(sinm_sb[64:128, lo:lo + 1024],
                                      sinm_sb[0:64, lo:lo + 1024])

            kt_rot = sb.tile([128, B * N], BF16)   # [batch b at b*N][n]
            qt_rot = sb.tile([128, B * N], BF16)
            vt_all = sb.tile([128, B * KC * LVW], BF16)
            # attn^T laid out [qc][b][row-block r][row-in-block]: shard
            # j = 4b+r of piece qc is the contiguous span (qc, b, r)
            attn_sb = sb.tile([128, B * N], BF16)
            attn4 = attn_sb.rearrange("p (q b r x) -> p q b r x", q=QC, b=B,
                                      x=128)

            # per-head ones columns of v-aug, set once
            nc.vector.memset(
                vt_all.rearrange("p (g e) -> p g e", e=65)[:, :, 64:65], 1.0)


            def proj_group(col0, b, sc):
                p = ps.tile([128, 512], F32, tag="b", name="pp")
                for hk in range(8):
                    nc.tensor.matmul(
                        p[:, :],
                        lhsT=wqk_sb[:, hk * 2 * LQK + col0:][:, :128],
                        rhs=xt_sb[:, (b * 8 + hk) * N + sc * 512:][:, :512],
                        start=(hk == 0),
                        stop=(hk == 7),
                    )
                return p

            def rotary_apply(psums, dst, pos0, W):
                """Rotary on a [128, W] span covering positions
                [pos0, pos0+W): stage psums to bf16 (DVE), then the
                partition-swapped sin multiply + cos multiply + add."""
                stage = sb.tile([128, 1024], BF16, tag="stg", bufs=3, name="stg")
                for i, p in enumerate(psums):
                    nc.vector.tensor_copy(stage[:, ts(i, 512)], p[:, :])
                tmp = sb.tile([128, 1024], BF16, tag="rta", bufs=3, name="rta")
                tmp2 = sb.tile([128, 1024], BF16, tag="rtb", bufs=3, name="rtb")
                sl = sinm_sb[:, pos0:pos0 + W]
                cl = cos2_sb[:, pos0:pos0 + W]
                for hh in (0, 64):
                    nc.vector.tensor_tensor(
                        tmp[hh:hh + 32, :W], stage[hh + 32:hh + 64, :W],
                        sl[hh + 32:hh + 64, :], MULT)
                    nc.vector.tensor_tensor(
                        tmp[hh + 32:hh + 64, :W], stage[hh:hh + 32, :W],
                        sl[hh:hh + 32, :], MULT)
                nc.vector.tensor_tensor(tmp2[:, :W], stage[:, :W], cl, MULT)
                nc.vector.tensor_tensor(dst, tmp2[:, :W], tmp[:, :W], ADD)

            def v_chunk(b, rc):
                p = ps.tile([128, LVW], F32, tag="b", name="vp")
                for hk in range(8):
                    nc.tensor.matmul(
                        p[:, :],
                        lhsT=xt_sb[:, (b * 8 + hk) * N + rc * 128:][:, :128],
                        rhs=wv_sb[:, ts(hk, LVW)],
                        start=(hk == 0),
                        stop=(hk == 7),
                    )
                nc.vector.tensor_copy(
                    vt_all[:, (b * KC + rc) * LVW:][:, :LVW].rearrange(
                        "p (h e) -> p h e", e=65)[:, :, 0:64],
                    p.rearrange("p (h e) -> p h e", e=65)[:, :, 0:64])

            def _av_mm(e, av0, av1, b, kc):
                base = (b * KC + kc) * LVW
                nc.tensor.matmul(
                    av0[:, :], lhsT=vt_all[:, base:][:, :65],
                    rhs=e[:, 0:512], start=(kc == 0), stop=(kc == KC - 1))
                nc.tensor.matmul(
                    av1[:, :], lhsT=vt_all[:, base + 65:][:, :65],
                    rhs=e[:, 512:1024], start=(kc == 0), stop=(kc == KC - 1))

            def finish_copy(av0, av1):
                # move attn_out^T (+denominator row 64) out of PSUM right
                # away so the next position's AV accumulation can reuse the
                # banks; also stage both denominator rows at partition 0 for
                # the broadcast matmul. Normalization itself runs later.
                a0 = sb.tile([65, 512], BF16, tag="avs", bufs=6, name="a0")
                a1 = sb.tile([65, 512], BF16, tag="avs", bufs=6, name="a1")
                nc.vector.tensor_copy(a0[:, :], av0[:, :])
                nc.vector.tensor_copy(a1[:, :], av1[:, :])
                ad = sb.tile([1, 1024], BF16, tag="adn", bufs=2, name="ad")
                nc.vector.tensor_copy(ad[:, 0:512], a0[64:65, :])
                nc.vector.tensor_copy(ad[:, 512:1024], a1[64:65, :])
                return a0, a1, ad

            def finish_norm(qc, b, a0, a1, ad):
                b_ps = ps.tile([128, 512], F32, tag="b", name="b_ps")
                nc.tensor.matmul(b_ps[0:64, :], lhsT=ones_sb[:, :],
                                 rhs=ad[:, 0:512], start=True, stop=True,
                                 tile_position=(0, 0))
                nc.tensor.matmul(b_ps[64:128, :], lhsT=ones_sb[:, :],
                                 rhs=ad[:, 512:1024], start=True, stop=True,
                                 tile_position=(0, 64))
                bd_sb = sb.tile([128, 512], F32, tag="bsd", bufs=2, name="bd_sb")
                nc.vector.tensor_copy(bd_sb[:, :], b_ps[:, :])
                b_sb = sb.tile([128, 512], F32, tag="bsb", bufs=2, name="b_sb")
                nc.vector.reciprocal_approx_fast(out=b_sb[:, :], in_=bd_sb[:, :])
                # both TT inputs must share a base partition: bring head B's
                # inv-denominators down to partitions 0-63
                b_lo = sb.tile([64, 512], F32, tag="blo", bufs=2, name="b_lo")
                nc.vector.tensor_copy(b_lo[:, :], b_sb[64:128, :])
                dst = attn4[:, qc, b, :, :]  # [128, 4, 128]
                b3 = b_sb.rearrange("p (j x) -> p j x", x=128)
                bl3 = b_lo.rearrange("p (j x) -> p j x", x=128)
                nc.vector.tensor_tensor(dst[0:64], a0[0:64, :].rearrange(
                    "p (j x) -> p j x", x=128), b3[0:64], MULT)
                nc.vector.tensor_tensor(dst[64:128], a1[0:64, :].rearrange(
                    "p (j x) -> p j x", x=128), bl3[:, :, :], MULT)

            def a2a_send(qc, b):
                # shard j=4b+r carries my 2 heads for (batch b, row block r)
                # of piece qc; b=None sends both batches in one DMA
                src = attn4[:, qc, :, :, :]          # [128, 2, 4, 128]
                d = a2a_in[qc].rearrange("(b r) p x -> p b r x", b=B)
                if b is not None:
                    src = src[:, b:b + 1, :, :]
                    d = d[:, b:b + 1, :, :]
                nc.sync.dma_start(d, src)

            def a2a_go(qc):
                nc.gpsimd.collective_compute(
                    "AllToAll", BYPASS, replica_groups=[list(range(8))],
                    ins=[a2a_in[qc].opt()], outs=[a2a_out[qc].opt()])

            def emit_a2a(qc):
                a2a_send(qc, None)
                a2a_go(qc)

            # tail-only: shard i of a2a_out = peer i's heads {2i, 2i+1} for
            # my 128 rows -> directly the outproj stationary operand
            def outproj_recv(qc):
                att_r = sb.tile([128, 8 * 128], BF16, tag="attr", bufs=2,
                                name="att_r")
                nc.gpsimd.dma_start(
                    att_r.rearrange("p (i x) -> p i x", i=8),
                    a2a_out[qc].rearrange("i p x -> p i x"))
                return att_r

            def outproj_mm(qc, att_r, nh):
                g3 = att_r.rearrange("p (c x) -> p c x", x=128)
                o_ps = ps.tile([128, 512], F32, tag="b", name="o_ps")
                for hc in range(8):
                    nc.tensor.matmul(
                        o_ps[:, :],
                        lhsT=g3[:, hc, :],
                        rhs=wout_sb[:, hc * H + nh * 512:][:, :512],
                        start=(hc == 0),
                        stop=(hc == 7),
                    )
                ob = sb.tile([128, 512], BF16, tag="ob", bufs=3, name="ob")
                nc.vector.tensor_copy(ob[:, :], o_ps[:, :])
                nc.sync.dma_start(out[qc, :, ts(nh, 512)], ob[:, :])

            def att_pos(qc, b, hooks):
                qt_p = qt_rot[:, b * N + qc * 512:][:, :512]
                av0 = ps.tile([65, 512], F32, tag="av", name="av0")
                av1 = ps.tile([65, 512], F32, tag="av", name="av1")
                exps = []
                for kc in range(KC):
                    s_ps = ps.tile([128, 1024], F32, tag="s", name="s_ps")
                    nc.tensor.matmul(
                        s_ps[:, 0:512],
                        lhsT=kt_rot[0:64, b * N + kc * 128:][:, :128],
                        rhs=qt_p[0:64, :], start=True, stop=True,
                        tile_position=(0, 0))
                    nc.tensor.matmul(
                        s_ps[:, 512:1024],
                        lhsT=kt_rot[64:128, b * N + kc * 128:][:, :128],
                        rhs=qt_p[64:128, :], start=True, stop=True,
                        tile_position=(64, 0))
                    e = sb.tile([128, 1024], BF16, tag="exp", bufs=8, name="e")
                    nc.scalar.activation(e[:, :], s_ps[:, :], EXP, scale=0.125)
                    exps.append(e)
                    for f in hooks.get(kc, []):
                        f()
                    if kc > 0:
                        _av_mm(exps[kc - 1], av0, av1, b, kc - 1)
                _av_mm(exps[KC - 1], av0, av1, b, KC - 1)
                return finish_copy(av0, av1)

            # ---- prologue ----
            # gate of the first exp: batch-0 K low half + Q sc0 + rotary;
            # V chunks 0-2 (b0) fill the PE while rotary runs on DVE
            k0a = [proj_group(GK, 0, 0), proj_group(GK, 0, 1)]
            rotary_apply(k0a, kt_rot[:, 0:1024], 0, 1024)
            q0s0 = [proj_group(GQ, 0, 0)]
            rotary_apply(q0s0, qt_rot[:, 0:512], 0, 512)
            for rc in range(3):
                v_chunk(0, rc)

            # closure helpers for hook tables
            def mk(f, *a):
                return lambda: f(*a)

            grabs = {}

            def grab(key, col0, b, sc):
                def g():
                    grabs.setdefault(key, []).append(proj_group(col0, b, sc))
                return g

            def rot(key, dst_tile, b, half):
                def g():
                    rotary_apply(
                        grabs.pop(key),
                        dst_tile[:, b * N + half * 1024:][:, :1024],
                        half * 1024, 1024)
                return g

            def rot_half(key, pos0):
                # 512-wide rotary chunk of batch-0 Q
                rotary_apply(grabs.pop(key),
                             qt_rot[:, pos0:pos0 + 512], pos0, 512)

            pend = {}

            def norm(qc, b):
                def g():
                    a0, a1, ad = pend.pop((qc, b))
                    finish_norm(qc, b, a0, a1, ad)
                return g

            def wout_dma():
                nc.sync.dma_start(
                    wout_sb.rearrange("p (k c) -> p k c", c=H),
                    wout.rearrange("(k p) c -> p k c", p=128))

            SEQ = [
                # (b, qc, hooks); norm(qc, b) finalizes the PREVIOUS
                # position; grab-pairs stay adjacent (2-buffer PSUM pool)
                (0, 0, dict(list({
                    0: [grab("k0b", GK, 0, 2)],
                    1: [grab("k0b", GK, 0, 3)],
                    2: [rot("k0b", kt_rot, 0, 1), mk(v_chunk, 0, 3)],
                    6: [grab("q0s1", GQ, 0, 1), mk(v_chunk, 0, 7)],
                    7: [mk(rot_half, "q0s1", 512), mk(v_chunk, 0, 8)],
                }.items()) + [(kc, [mk(v_chunk, 0, kc + 1)])
                              for kc in (3, 4, 5)]
                             + [(kc, [mk(v_chunk, 0, kc + 1)])
                                for kc in range(8, 15)])),
                (0, 1, {
                    1: [grab("k1a", GK, 1, 0)],
                    3: [grab("k1a", GK, 1, 1)],
                    5: [rot("k1a", kt_rot, 1, 0)],
                    6: [norm(0, 0), wout_dma],
                    7: [grab("q1a", GQ, 1, 0)],
                    9: [grab("q1a", GQ, 1, 1)],
                    11: [rot("q1a", qt_rot, 1, 0)],
                    12: [mk(v_chunk, 1, 0), mk(v_chunk, 1, 1),
                         mk(v_chunk, 1, 2)],
                    13: [mk(v_chunk, 1, 3), mk(v_chunk, 1, 4),
                         mk(v_chunk, 1, 5)],
                    14: [grab("k1b", GK, 1, 2)],
                    15: [grab("k1b", GK, 1, 3)],
                }),
                (1, 0, dict(list({
                    0: [rot("k1b", kt_rot, 1, 1)],
                    2: [norm(1, 0)],
                    3: [grab("q0b", GQ, 0, 2)],
                    4: [grab("q0b", GQ, 0, 3)],
                    5: [rot("q0b", qt_rot, 0, 1), mk(v_chunk, 1, 6)],
                }.items()) + [(kc, [mk(v_chunk, 1, kc + 1)])
                              for kc in range(6, 15)])),
                (0, 2, {
                    1: [grab("q1b", GQ, 1, 2)],
                    3: [grab("q1b", GQ, 1, 3)],
                    5: [rot("q1b", qt_rot, 1, 1)],
                    6: [norm(0, 1)],
                    8: [mk(emit_a2a, 0)],
                }),
                (1, 1, {
                    2: [norm(2, 0)],
                }),
                (0, 3, {
                    2: [norm(1, 1)],
                    6: [mk(emit_a2a, 1)],
                }),
                (1, 2, {
                    2: [norm(3, 0)],
                }),
                (1, 3, {
                    2: [norm(2, 1)],
                    4: [mk(emit_a2a, 2)],
                    8: [mk(a2a_send, 3, 0)],
                }),
            ]
            for b, qc, hooks in SEQ:
                pend[(qc, b)] = att_pos(qc, b, hooks)

            # tail: final normalization + piece-3 collective, then ALL
            # output projections (priority-pinned so the scheduler cannot
            # hoist collective-gated work into the attention stream)
            with tc.high_priority():
                norm(3, 1)()
                a2a_send(3, 1)
            a2a_go(3)
            with tc.high_priority(offset=-10_000_000):
                for qc in range(QC):
                    ar = outproj_recv(qc)
                    outproj_mm(qc, ar, 0)
                    outproj_mm(qc, ar, 1)

    nc.finalize()
    return nc


_NC = None


def _get_nc():
    global _NC
    if _NC is None:
        _NC = build_nc()
    return _NC


def _bf16(a):
    return np.ascontiguousarray(a.astype(ml_dtypes.bfloat16))


def make_in_maps(x, rotary_emb, w_qkv, w_out):
    x = np.asarray(x, np.float32)
    rotary_emb = np.asarray(rotary_emb, np.float32)
    w_qkv = np.asarray(w_qkv, np.float32)
    w_out = np.asarray(w_out, np.float32)
    cosT = np.cos(rotary_emb).T.astype(np.float32)  # [64, N]
    sinT = np.sin(rotary_emb).T.astype(np.float32)
    sswp = np.concatenate([sinT[32:], -sinT[:32]], axis=0)
    cos2_a = _bf16(cosT)
    sinm_a = _bf16(sswp)
    wout_bf = _bf16(w_out)
    xT_a = _bf16(np.concatenate([x[0].T, x[1].T], axis=1))  # [H, B*N]
    in_maps = []
    for c in range(NC_):
        h0 = LH * c  # heads {2c, 2c+1}
        wq_loc = w_qkv[:, 64 * h0: 64 * h0 + LQK]
        wk_loc = w_qkv[:, H + 64 * h0: H + 64 * h0 + LQK]
        wv_loc = w_qkv[:, 2 * H + 64 * h0: 2 * H + 64 * h0 + LQK]
        wv_aug = np.zeros((H, LVW), np.float32)
        for j in range(LH):
            wv_aug[:, 65 * j: 65 * j + 64] = wv_loc[:, 64 * j: 64 * j + 64]
        # column-grouped wqk: [K | Q]
        wqk_g = np.concatenate([wk_loc, wq_loc], axis=1)
        in_maps.append({
            "xT": xT_a,
            "wqk": _bf16(wqk_g),
            "wv": _bf16(wv_aug),
            "wout": wout_bf,
            "cos2": cos2_a,
            "sinm": sinm_a,
        })
    return in_maps


def run(x, rotary_emb, w_qkv, w_out, trace=False, tmpdir=None):
    nc = _get_nc()
    in_maps = make_in_maps(x, rotary_emb, w_qkv, w_out)
    res = run_bass_kernel_spmd(nc, in_maps, list(range(NC_)), trace=trace,
                               tmpdir=tmpdir)
    full = np.empty((B, N, H), np.float32)
    for c in range(NC_):
        b, r = c // 4, c % 4
        piece = np.asarray(res.results[c]["out"]).astype(np.float32)
        for qc in range(QC):
            full[b, 512 * qc + 128 * r: 512 * qc + 128 * r + 128] = piece[qc]
    return full, res


def kernel(x, rotary_emb, w_qkv, w_out):
    full, _ = run(x, rotary_emb, w_qkv, w_out)
    return full
